# revision 29
# baseline (speedup 1.0000x reference)
"""Bass/Trainium2 kernel for nn_DFTLayer: out[b,f,k] = DFT_1024(x[b,f,:]).

reference: real = einsum('bfs,ks->bfk', x, wcos); imag = ... wsin
           out  = complex(real, -imag),  x: [16, 1024, 1024] f32.

Strategy (8 NeuronCores, data-parallel over batch, 2 batches/core):
  - Hermitian symmetry (x real): out[k] = conj(out[N-k]) -> device computes
    k = 1..512 only; col 0 is a host row-sum, cols 513..1023 a host mirror.
  - TWO levels of cosine/sine parity folds (fast-DCT style), all on host:
      level 1: u[s] = x[s]+x[N-s], v'[s] = -(x[s]-x[N-s]),  s = 0..511
      level 2 (split k by parity, fold s about 256):
        u2 = u[s]+u[512-s], uo = u[s]-u[512-s]   (re even / re odd)
        ve = v'[s]-v'[512-s], vo = v'[s]+v'[512-s] (im even / im odd)
    Device work: 4 transforms [2048,256]x[256,256] per core -> PE cycles
    drop 4x vs the naive half-spectrum GEMM. Edge terms ((-1)^k x[512],
    (-1)^m u[256], (-1)^m v'[256]) applied on host.
  - The 16 kernel chunks [128,128] are the PE-stationary operands (reused
    across all f); folded data streams as the N=512 moving operand, so
    LDWEIGHTS fully hides under the 213ns matmul stream. Output lands
    transposed [m, f] in PSUM; host de-transposes (free). Per f-block of
    512 rows: 16 matmuls -> 8 PSUM banks -> ACT/DVE alternating bf16
    copies -> one 1MB store.
  - ALL DMA (inputs then stores) goes through the single sync HWDGE ring:
    strict FIFO means stores never steal HBM bandwidth from the input
    stream the PE is waiting on. Inputs are packed in ONE DRAM tensor in
    consumption order. Dummy matmuls warm the PE (HAM 1.2->2.4 GHz)
    during the DMA head.
  - The cos/sin tables are GENERATED on device (iota -> s*k int product,
    int32 mask range-reduction, ACT Sin) -- zero DMA bytes for them.
  - bf16 I/O: 8MB/core round trip (HBM-bound), rel err ~3e-3 << 2e-2.
"""

import sys

for _p in ("/opt/trn_rl_repo", "/root/.axon_site/_ro/trn_rl_repo"):
    if _p not in sys.path:
        sys.path.append(_p)

import numpy as np
import ml_dtypes
from contextlib import ExitStack

BF16 = ml_dtypes.bfloat16

N_CORES = 8
B, F_FULL, S = 16, 1024, 1024          # x: [B, F_FULL, S]
F = (B // N_CORES) * F_FULL            # 2048 rows per core
KD = 512                               # freq cols 1..512 (even|odd packed)
SH = 256                               # level-2 contraction length
FB = 4                                 # f-blocks per core (512 rows each)
NWARM = 14                             # PE warm-up matmuls (HAM un-throttle)

OFF_D = 0                              # data: fb*4096 + t*1024 + c*512
INP_W = FB * 4096                      # DFT kernels are generated on-device

_CACHE = {}


def _build():
    """Build + compile the per-core Bass program (cached)."""
    if "nc" in _CACHE:
        return _CACHE["nc"]

    from concourse import bacc, tile, mybir

    f32 = mybir.dt.float32
    bf16 = mybir.dt.bfloat16

    nc = bacc.Bacc("TRN2", target_bir_lowering=False, debug=False)

    inp_d = nc.dram_tensor("inp", [128, INP_W], bf16, kind="ExternalInput")
    # transposed outputs: row = t*256 + m  (t: u2,uo,ve,vo), col = f
    od_d = nc.dram_tensor("od", [4 * SH, F], bf16, kind="ExternalOutput")
    od_r = od_d[:].rearrange("(g m) f -> m g f", g=8)   # g = t*2 + mh

    with tile.TileContext(nc) as tc, ExitStack() as ctx:
        wpool = ctx.enter_context(tc.tile_pool(name="w", bufs=1))
        opool = ctx.enter_context(tc.tile_pool(name="o", bufs=3))
        ptpool = ctx.enter_context(tc.tile_pool(name="pt", bufs=8, space="PSUM"))

        inp_t = wpool.tile([128, INP_W], bf16, tag="inp")

        # input DMA chunks, in consumption order, on the sync HWDGE ring
        def load(lo, hi):
            nc.sync.dma_start(inp_t[:, lo:hi], inp_d[:, lo:hi])

        load(0, 1024)                            # fb0 u2 data
        load(1024, 2048)                         # fb0 uo data
        load(2048, 4096)                         # fb0 ve, vo data
        for fb in range(1, FB):
            load(fb * 4096, (fb + 1) * 4096)

        # PE warm-up on a zeroed tile; PSUM target is buffer 0 of the pool
        # (recycled by real matmuls via WAW dependency).
        warm_t = wpool.tile([128, 512], bf16, tag="warm")
        nc.gpsimd.memset(warm_t[:], 0)
        pw = ptpool.tile([128, 512], f32, tag="ps", name="pswarm")
        for _ in range(NWARM):
            nc.tensor.matmul(pw[:, 0:256], warm_t[:, 0:128], warm_t[:, 0:256],
                             start=True, stop=True)

        # ---- generate the DFT kernel tables on device (no DMA) ----
        # wg[p, t*512 + c*256 + col] with col = mh*128 + m:
        #   t=0: cos(2*pi*s*me/512)    t=1: cos(2*pi*s*(2m+1)/1024)
        #   t=2: sin(2*pi*s*me/512)    t=3: sin(2*pi*s*(2m+1)/1024)
        # where s = c*128 + p, me = col+1. All in f32, all exact: h = s*kk/P
        # (integers < 2^17 scaled by a power of two), +0.25 turns sin into
        # cos, n = (h + 2^23) - 2^23 rounds to nearest, f = h - n is in
        # [-1/2, 1/2], value = Sin(2*pi*f).
        f32 = mybir.dt.float32
        SinF = mybir.ActivationFunctionType.Sin
        PI = 3.14159265358979323846
        TWO23 = float(2 ** 23)
        wg_t = wpool.tile([128, 2048], bf16, tag="wg")
        r_e = wpool.tile([128, 256], f32, tag="re")
        nc.gpsimd.iota(r_e[:], [[1, 256]], base=1, channel_multiplier=0,
                       allow_small_or_imprecise_dtypes=True)
        r_o = wpool.tile([128, 256], f32, tag="ro")
        nc.gpsimd.iota(r_o[:], [[2, 256]], base=1, channel_multiplier=0,
                       allow_small_or_imprecise_dtypes=True)
        sf = wpool.tile([128, 2], f32, tag="sf")
        nc.gpsimd.iota(sf[:], [[128, 2]], base=0, channel_multiplier=1,
                       allow_small_or_imprecise_dtypes=True)
        cst = wpool.tile([128, 2], f32, tag="cst")   # 2*pi, scratch
        nc.gpsimd.memset(cst[:, 0:1], 2.0 * PI)
        nc.gpsimd.memset(cst[:, 1:2], 0.0)
        # dummy activation: hoists the ACT Sin table load into the DMA head
        dum = wpool.tile([128, 1], bf16, tag="dum")
        nc.scalar.activation(dum[:], cst[:, 1:2], SinF)
        qpool = ctx.enter_context(tc.tile_pool(name="q", bufs=4))
        hs = {}
        for par in range(2):                 # 0: even(P=512), 1: odd(P=1024)
            P = 512.0 if par == 0 else 1024.0
            ramp = r_e if par == 0 else r_o
            for c in range(2):
                h_t = qpool.tile([128, 256], f32, tag="h", name=f"h{par}_{c}")
                nc.vector.tensor_scalar(h_t[:], ramp[:], sf[:, c:c + 1],
                                        1.0 / P, op0=mybir.AluOpType.mult,
                                        op1=mybir.AluOpType.mult)
                hs[(par, c)] = h_t
        for t in range(4):
            par = t % 2                      # t0/t2 even, t1/t3 odd
            for c in range(2):
                h_t = hs[(par, c)]
                if t < 2:                    # cosine: shift by +1/4 period
                    h2 = qpool.tile([128, 256], f32, tag="h2",
                                    name=f"h2{t}_{c}")
                    nc.vector.tensor_scalar_add(h2[:], h_t[:], 0.25)
                    h_t = h2
                n_t = qpool.tile([128, 256], f32, tag="n", name=f"n{t}_{c}")
                nc.vector.tensor_scalar(n_t[:], h_t[:], TWO23, -TWO23,
                                        op0=mybir.AluOpType.add,
                                        op1=mybir.AluOpType.add)
                f_t = qpool.tile([128, 256], f32, tag="f", name=f"f{t}_{c}")
                nc.gpsimd.tensor_sub(f_t[:], h_t[:], n_t[:])
                lo = t * 512 + c * 256
                nc.scalar.activation(wg_t[:, lo:lo + 256], f_t[:], SinF,
                                     scale=cst[:, 0:1])

        def w_sl(t, c, mh):
            lo = t * 512 + c * 256 + mh * 128
            return wg_t[:, lo:lo + 128]

        def d_sl(fb, t, c):
            lo = OFF_D + fb * 4096 + t * 1024 + c * 512
            return inp_t[:, lo:lo + 512]

        for fb in range(FB):
            o_t = opool.tile([128, 8, 512], bf16, tag="o", name=f"o{fb}")
            for g in range(8):
                t, mh = g // 2, g % 2
                ps = ptpool.tile([128, 512], f32, tag="ps", name=f"ps{fb}_{g}")
                for c in range(2):
                    nc.tensor.matmul(ps[:], w_sl(t, c, mh), d_sl(fb, t, c),
                                     start=(c == 0), stop=(c == 1))
                # split every copy across both engines: halves the latency
                # until the PSUM bank frees and until the store can start
                nc.scalar.copy(o_t[:, g, 0:256], ps[:, 0:256])
                nc.vector.tensor_copy(o_t[:, g, 256:512], ps[:, 256:512])
            fsl = slice(fb * 512, (fb + 1) * 512)
            if fb < FB - 1:
                nc.sync.dma_start(od_r[:, :, fsl], o_t[:])      # 1MB store
            else:
                # tail: store in 3 pieces as copies complete
                nc.sync.dma_start(od_r[:, 0:4, fsl], o_t[:, 0:4, :])
                nc.sync.dma_start(od_r[:, 4:7, fsl], o_t[:, 4:7, :])
                nc.sync.dma_start(od_r[:, 7:8, fsl], o_t[:, 7:8, :])

    nc.compile()
    _CACHE["nc"] = nc
    return nc


def _pack_data(a):
    """[F=2048, SH=256] row-major -> [128, FB, 2, 512] rhs payloads.

    out[p, fb, c, f] = a[fb*512 + f, c*128 + p]
    """
    t = a.reshape(FB, 512, 2, 128)                # [fb, f, c, p]
    return t.transpose(3, 0, 2, 1)


def kernel(x, wsin, wcos):
    from concourse.bass_utils import run_bass_kernel_spmd

    x = np.asarray(x, dtype=np.float32)

    nc = _build()

    # ---- host folds (f32, exact) ----
    xr = x.reshape(B, F_FULL, S)
    rev = xr[:, :, :512:-1]                   # cols 1023..513  (s' = 1024-s)
    u = np.empty((B, F_FULL, 512), dtype=np.float32)
    v = np.empty((B, F_FULL, 512), dtype=np.float32)   # v' = -(x[s]-x[N-s])
    u[:, :, 0] = xr[:, :, 0]
    v[:, :, 0] = -xr[:, :, 0]
    u[:, :, 1:] = xr[:, :, 1:512] + rev
    np.subtract(rev, xr[:, :, 1:512], out=v[:, :, 1:])

    urev = u[:, :, 511:256:-1]                # u[512-s], s = 1..255
    vrev = v[:, :, 511:256:-1]
    tf = np.empty((4, B, F_FULL, SH), dtype=np.float32)  # u2, uo, ve, vo
    tf[0, :, :, 0] = u[:, :, 0]
    tf[1, :, :, 0] = u[:, :, 0]
    tf[2, :, :, 0] = 0.0
    tf[3, :, :, 0] = v[:, :, 0]
    tf[0, :, :, 1:] = u[:, :, 1:256] + urev
    tf[1, :, :, 1:] = u[:, :, 1:256] - urev
    tf[2, :, :, 1:] = v[:, :, 1:256] - vrev
    tf[3, :, :, 1:] = v[:, :, 1:256] + vrev
    eu = u[:, :, 256].copy()                  # edge terms (host-applied)
    ev = v[:, :, 256].copy()
    tf16 = tf.astype(BF16)

    bpc = B // N_CORES
    in_maps = []
    for cc in range(N_CORES):
        sl = slice(cc * bpc, (cc + 1) * bpc)
        D = np.stack([_pack_data(tf16[t, sl].reshape(F, SH)) for t in range(4)],
                     axis=2)                  # [p, fb, t, c, f]
        in_maps.append({"inp": np.ascontiguousarray(D.reshape(128, FB * 4096))})

    res = run_bass_kernel_spmd(
        nc, in_maps, core_ids=list(range(N_CORES)), **_CACHE.get("run_kwargs", {})
    )
    kernel.last_results = res

    alt_e = np.where(np.arange(1, 257) % 2 == 0, np.float32(1), np.float32(-1))
    alt_o = np.where(np.arange(256) % 2 == 0, np.float32(1), np.float32(-1))
    out = np.empty((B, F_FULL, S), dtype=np.complex64)
    fv = out.view(np.float32).reshape(B, F_FULL, 2 * S)
    for cc in range(N_CORES):
        b0 = cc * bpc
        od = np.asarray(res.results[cc]["od"]).astype(np.float32)
        # od[t*256 + m, f] -> o[bpc, F_FULL, t, m]
        o = np.ascontiguousarray(od.reshape(4, 256, F).transpose(2, 0, 1))
        o = o.reshape(bpc, F_FULL, 4, 256)
        # edge terms: re_e += (-1)^me * u[256],  im_o += (-1)^mo * v'[256]
        o[:, :, 0, :] += eu[b0:b0 + bpc, :, None] * alt_e
        o[:, :, 3, :] += ev[b0:b0 + bpc, :, None] * alt_o
        re = np.empty((bpc, F_FULL, KD), dtype=np.float32)
        im = np.empty((bpc, F_FULL, KD), dtype=np.float32)   # already -imag
        re[:, :, 1::2] = o[:, :, 0, :]        # k even
        re[:, :, 0::2] = o[:, :, 1, :]        # k odd
        im[:, :, 1::2] = o[:, :, 2, :]
        im[:, :, 0::2] = o[:, :, 3, :]
        blk = fv[b0:b0 + bpc]
        # col 0: real = row-sum of x (cos(0)=1), imag = 0 (sin(0)=0)
        blk[:, :, 0] = x[b0:b0 + bpc].sum(axis=-1, dtype=np.float32)
        blk[:, :, 1] = 0.0
        blk[:, :, 2:2 * KD + 2:2] = re          # real, k = 1..512
        blk[:, :, 3:2 * KD + 3:2] = im          # imag, k = 1..512
        # Hermitian mirror: out[k] = conj(out[1024-k]) for k = 513..1023
        blk[:, :, 2 * KD + 2::2] = re[:, :, KD - 2::-1]
        blk[:, :, 2 * KD + 3::2] = -im[:, :, KD - 2::-1]
    # the s = 512 fold edge term: real[k] += (-1)^k * x[:, :, 512]
    alt = np.where(np.arange(1, S) % 2 == 0, np.float32(1.0), np.float32(-1.0))
    fv[:, :, 2::2] += x[:, :, 512:513] * alt[None, None, :]
    return out


# revision 30
# speedup vs baseline: 1.0005x; 1.0005x over previous
"""Bass/Trainium2 kernel for nn_DFTLayer: out[b,f,k] = DFT_1024(x[b,f,:]).

reference: real = einsum('bfs,ks->bfk', x, wcos); imag = ... wsin
           out  = complex(real, -imag),  x: [16, 1024, 1024] f32.

Strategy (8 NeuronCores, data-parallel over batch, 2 batches/core):
  - Hermitian symmetry (x real): out[k] = conj(out[N-k]) -> device computes
    k = 1..512 only; col 0 is a host row-sum, cols 513..1023 a host mirror.
  - TWO levels of cosine/sine parity folds (fast-DCT style), all on host:
      level 1: u[s] = x[s]+x[N-s], v'[s] = -(x[s]-x[N-s]),  s = 0..511
      level 2 (split k by parity, fold s about 256):
        u2 = u[s]+u[512-s], uo = u[s]-u[512-s]   (re even / re odd)
        ve = v'[s]-v'[512-s], vo = v'[s]+v'[512-s] (im even / im odd)
    Device work: 4 transforms [2048,256]x[256,256] per core -> PE cycles
    drop 4x vs the naive half-spectrum GEMM. Edge terms ((-1)^k x[512],
    (-1)^m u[256], (-1)^m v'[256]) applied on host.
  - The 16 kernel chunks [128,128] are the PE-stationary operands (reused
    across all f); folded data streams as the N=512 moving operand, so
    LDWEIGHTS fully hides under the 213ns matmul stream. Output lands
    transposed [m, f] in PSUM; host de-transposes (free). Per f-block of
    512 rows: 16 matmuls -> 8 PSUM banks -> ACT/DVE alternating bf16
    copies -> one 1MB store.
  - ALL DMA (inputs then stores) goes through the single sync HWDGE ring:
    strict FIFO means stores never steal HBM bandwidth from the input
    stream the PE is waiting on. Inputs are packed in ONE DRAM tensor in
    consumption order. Dummy matmuls warm the PE (HAM 1.2->2.4 GHz)
    during the DMA head.
  - The cos/sin tables are GENERATED on device (f32 iota -> exact h=s*k/P,
    round-to-nearest via +-2^23, ACT Sin(2*pi*(h-n))) -- zero DMA bytes.
  - bf16 I/O: 8MB/core round trip (HBM-bound), rel err ~3e-3 << 2e-2.
"""

import sys

for _p in ("/opt/trn_rl_repo", "/root/.axon_site/_ro/trn_rl_repo"):
    if _p not in sys.path:
        sys.path.append(_p)

import numpy as np
import ml_dtypes
from contextlib import ExitStack

BF16 = ml_dtypes.bfloat16

N_CORES = 8
B, F_FULL, S = 16, 1024, 1024          # x: [B, F_FULL, S]
F = (B // N_CORES) * F_FULL            # 2048 rows per core
KD = 512                               # freq cols 1..512 (even|odd packed)
SH = 256                               # level-2 contraction length
FB = 4                                 # f-blocks per core (512 rows each)
NWARM = 14                             # PE warm-up matmuls (HAM un-throttle)

OFF_D = 0                              # data: fb*4096 + t*1024 + c*512
INP_W = FB * 4096                      # DFT kernels are generated on-device

_CACHE = {}


def _build():
    """Build + compile the per-core Bass program (cached)."""
    if "nc" in _CACHE:
        return _CACHE["nc"]

    from concourse import bacc, tile, mybir

    f32 = mybir.dt.float32
    bf16 = mybir.dt.bfloat16

    nc = bacc.Bacc("TRN2", target_bir_lowering=False, debug=False)

    inp_d = nc.dram_tensor("inp", [128, INP_W], bf16, kind="ExternalInput")
    # transposed outputs: row = t*256 + m  (t: u2,uo,ve,vo), col = f
    od_d = nc.dram_tensor("od", [4 * SH, F], bf16, kind="ExternalOutput")
    od_r = od_d[:].rearrange("(g m) f -> m g f", g=8)   # g = t*2 + mh

    with tile.TileContext(nc) as tc, ExitStack() as ctx:
        wpool = ctx.enter_context(tc.tile_pool(name="w", bufs=1))
        opool = ctx.enter_context(tc.tile_pool(name="o", bufs=3))
        ptpool = ctx.enter_context(tc.tile_pool(name="pt", bufs=8, space="PSUM"))

        inp_t = wpool.tile([128, INP_W], bf16, tag="inp")

        # input DMA chunks, in consumption order, on the sync HWDGE ring
        def load(lo, hi):
            nc.sync.dma_start(inp_t[:, lo:hi], inp_d[:, lo:hi])

        load(0, 1024)                            # fb0 u2 data
        load(1024, 2048)                         # fb0 uo data
        load(2048, 4096)                         # fb0 ve, vo data
        for fb in range(1, FB):
            load(fb * 4096, (fb + 1) * 4096)

        # PE warm-up on a zeroed tile; PSUM target is buffer 0 of the pool
        # (recycled by real matmuls via WAW dependency).
        warm_t = wpool.tile([128, 512], bf16, tag="warm")
        nc.gpsimd.memset(warm_t[:], 0)
        pw = ptpool.tile([128, 512], f32, tag="ps", name="pswarm")
        for _ in range(NWARM):
            nc.tensor.matmul(pw[:, 0:256], warm_t[:, 0:128], warm_t[:, 0:256],
                             start=True, stop=True)

        # ---- generate the DFT kernel tables on device (no DMA) ----
        # wg[p, t*512 + c*256 + col] with col = mh*128 + m:
        #   t=0: cos(2*pi*s*me/512)    t=1: cos(2*pi*s*(2m+1)/1024)
        #   t=2: sin(2*pi*s*me/512)    t=3: sin(2*pi*s*(2m+1)/1024)
        # where s = c*128 + p, me = col+1. All in f32, all exact: h = s*kk/P
        # (integers < 2^17 scaled by a power of two), +0.25 turns sin into
        # cos, n = (h + 2^23) - 2^23 rounds to nearest, f = h - n is in
        # [-1/2, 1/2], value = Sin(2*pi*f).
        f32 = mybir.dt.float32
        SinF = mybir.ActivationFunctionType.Sin
        PI = 3.14159265358979323846
        TWO23 = float(2 ** 23)
        wg_t = wpool.tile([128, 2048], bf16, tag="wg")
        r_e = wpool.tile([128, 256], f32, tag="re")
        nc.gpsimd.iota(r_e[:], [[1, 256]], base=1, channel_multiplier=0,
                       allow_small_or_imprecise_dtypes=True)
        r_o = wpool.tile([128, 256], f32, tag="ro")
        nc.gpsimd.iota(r_o[:], [[2, 256]], base=1, channel_multiplier=0,
                       allow_small_or_imprecise_dtypes=True)
        sf = wpool.tile([128, 2], f32, tag="sf")
        nc.gpsimd.iota(sf[:], [[128, 2]], base=0, channel_multiplier=1,
                       allow_small_or_imprecise_dtypes=True)
        cst = wpool.tile([128, 2], f32, tag="cst")   # 2*pi, scratch
        nc.gpsimd.memset(cst[:, 0:1], 2.0 * PI)
        nc.gpsimd.memset(cst[:, 1:2], 0.0)
        # dummy activation: hoists the ACT Sin table load into the DMA head
        dum = wpool.tile([128, 1], bf16, tag="dum")
        nc.scalar.activation(dum[:], cst[:, 1:2], SinF)
        qpool = ctx.enter_context(tc.tile_pool(name="q", bufs=4))
        hs = {}
        for par in range(2):                 # 0: even(P=512), 1: odd(P=1024)
            P = 512.0 if par == 0 else 1024.0
            ramp = r_e if par == 0 else r_o
            for c in range(2):
                h_t = qpool.tile([128, 256], f32, tag="h", name=f"h{par}_{c}")
                nc.vector.tensor_scalar(h_t[:], ramp[:], sf[:, c:c + 1],
                                        1.0 / P, op0=mybir.AluOpType.mult,
                                        op1=mybir.AluOpType.mult)
                hs[(par, c)] = h_t
        for t in range(4):
            par = t % 2                      # t0/t2 even, t1/t3 odd
            for c in range(2):
                h_t = hs[(par, c)]
                if t < 2:                    # cosine: shift by +1/4 period
                    h2 = qpool.tile([128, 256], f32, tag="h2",
                                    name=f"h2{t}_{c}")
                    nc.vector.tensor_scalar_add(h2[:], h_t[:], 0.25)
                    h_t = h2
                n_t = qpool.tile([128, 256], f32, tag="n", name=f"n{t}_{c}")
                nc.vector.tensor_scalar(n_t[:], h_t[:], TWO23, -TWO23,
                                        op0=mybir.AluOpType.add,
                                        op1=mybir.AluOpType.add)
                f_t = qpool.tile([128, 256], f32, tag="f", name=f"f{t}_{c}")
                nc.gpsimd.tensor_sub(f_t[:], h_t[:], n_t[:])
                lo = t * 512 + c * 256
                nc.scalar.activation(wg_t[:, lo:lo + 256], f_t[:], SinF,
                                     scale=cst[:, 0:1])

        def w_sl(t, c, mh):
            lo = t * 512 + c * 256 + mh * 128
            return wg_t[:, lo:lo + 128]

        def d_sl(fb, t, c):
            lo = OFF_D + fb * 4096 + t * 1024 + c * 512
            return inp_t[:, lo:lo + 512]

        for fb in range(FB):
            o_t = opool.tile([128, 8, 512], bf16, tag="o", name=f"o{fb}")
            for g in range(8):
                t, mh = g // 2, g % 2
                ps = ptpool.tile([128, 512], f32, tag="ps", name=f"ps{fb}_{g}")
                for c in range(2):
                    nc.tensor.matmul(ps[:], w_sl(t, c, mh), d_sl(fb, t, c),
                                     start=(c == 0), stop=(c == 1))
                # split every copy across both engines: halves the latency
                # until the PSUM bank frees and until the store can start
                nc.scalar.copy(o_t[:, g, 0:256], ps[:, 0:256])
                nc.vector.tensor_copy(o_t[:, g, 256:512], ps[:, 256:512])
            fsl = slice(fb * 512, (fb + 1) * 512)
            if fb < FB - 1:
                nc.sync.dma_start(od_r[:, :, fsl], o_t[:])      # 1MB store
            else:
                # tail: store in 3 pieces as copies complete
                nc.sync.dma_start(od_r[:, 0:4, fsl], o_t[:, 0:4, :])
                nc.sync.dma_start(od_r[:, 4:7, fsl], o_t[:, 4:7, :])
                nc.sync.dma_start(od_r[:, 7:8, fsl], o_t[:, 7:8, :])

    nc.compile()
    _CACHE["nc"] = nc
    return nc


def _pack_data(a):
    """[F=2048, SH=256] row-major -> [128, FB, 2, 512] rhs payloads.

    out[p, fb, c, f] = a[fb*512 + f, c*128 + p]
    """
    t = a.reshape(FB, 512, 2, 128)                # [fb, f, c, p]
    return t.transpose(3, 0, 2, 1)


def kernel(x, wsin, wcos):
    from concourse.bass_utils import run_bass_kernel_spmd

    x = np.asarray(x, dtype=np.float32)

    nc = _build()

    # ---- host folds (f32, exact) ----
    xr = x.reshape(B, F_FULL, S)
    rev = xr[:, :, :512:-1]                   # cols 1023..513  (s' = 1024-s)
    u = np.empty((B, F_FULL, 512), dtype=np.float32)
    v = np.empty((B, F_FULL, 512), dtype=np.float32)   # v' = -(x[s]-x[N-s])
    u[:, :, 0] = xr[:, :, 0]
    v[:, :, 0] = -xr[:, :, 0]
    u[:, :, 1:] = xr[:, :, 1:512] + rev
    np.subtract(rev, xr[:, :, 1:512], out=v[:, :, 1:])

    urev = u[:, :, 511:256:-1]                # u[512-s], s = 1..255
    vrev = v[:, :, 511:256:-1]
    tf = np.empty((4, B, F_FULL, SH), dtype=np.float32)  # u2, uo, ve, vo
    tf[0, :, :, 0] = u[:, :, 0]
    tf[1, :, :, 0] = u[:, :, 0]
    tf[2, :, :, 0] = 0.0
    tf[3, :, :, 0] = v[:, :, 0]
    tf[0, :, :, 1:] = u[:, :, 1:256] + urev
    tf[1, :, :, 1:] = u[:, :, 1:256] - urev
    tf[2, :, :, 1:] = v[:, :, 1:256] - vrev
    tf[3, :, :, 1:] = v[:, :, 1:256] + vrev
    eu = u[:, :, 256].copy()                  # edge terms (host-applied)
    ev = v[:, :, 256].copy()
    tf16 = tf.astype(BF16)

    bpc = B // N_CORES
    in_maps = []
    for cc in range(N_CORES):
        sl = slice(cc * bpc, (cc + 1) * bpc)
        D = np.stack([_pack_data(tf16[t, sl].reshape(F, SH)) for t in range(4)],
                     axis=2)                  # [p, fb, t, c, f]
        in_maps.append({"inp": np.ascontiguousarray(D.reshape(128, FB * 4096))})

    res = run_bass_kernel_spmd(
        nc, in_maps, core_ids=list(range(N_CORES)), **_CACHE.get("run_kwargs", {})
    )
    kernel.last_results = res

    alt_e = np.where(np.arange(1, 257) % 2 == 0, np.float32(1), np.float32(-1))
    alt_o = np.where(np.arange(256) % 2 == 0, np.float32(1), np.float32(-1))
    out = np.empty((B, F_FULL, S), dtype=np.complex64)
    fv = out.view(np.float32).reshape(B, F_FULL, 2 * S)
    for cc in range(N_CORES):
        b0 = cc * bpc
        od = np.asarray(res.results[cc]["od"]).astype(np.float32)
        # od[t*256 + m, f] -> o[bpc, F_FULL, t, m]
        o = np.ascontiguousarray(od.reshape(4, 256, F).transpose(2, 0, 1))
        o = o.reshape(bpc, F_FULL, 4, 256)
        # edge terms: re_e += (-1)^me * u[256],  im_o += (-1)^mo * v'[256]
        o[:, :, 0, :] += eu[b0:b0 + bpc, :, None] * alt_e
        o[:, :, 3, :] += ev[b0:b0 + bpc, :, None] * alt_o
        re = np.empty((bpc, F_FULL, KD), dtype=np.float32)
        im = np.empty((bpc, F_FULL, KD), dtype=np.float32)   # already -imag
        re[:, :, 1::2] = o[:, :, 0, :]        # k even
        re[:, :, 0::2] = o[:, :, 1, :]        # k odd
        im[:, :, 1::2] = o[:, :, 2, :]
        im[:, :, 0::2] = o[:, :, 3, :]
        blk = fv[b0:b0 + bpc]
        # col 0: real = row-sum of x (cos(0)=1), imag = 0 (sin(0)=0)
        blk[:, :, 0] = x[b0:b0 + bpc].sum(axis=-1, dtype=np.float32)
        blk[:, :, 1] = 0.0
        blk[:, :, 2:2 * KD + 2:2] = re          # real, k = 1..512
        blk[:, :, 3:2 * KD + 3:2] = im          # imag, k = 1..512
        # Hermitian mirror: out[k] = conj(out[1024-k]) for k = 513..1023
        blk[:, :, 2 * KD + 2::2] = re[:, :, KD - 2::-1]
        blk[:, :, 2 * KD + 3::2] = -im[:, :, KD - 2::-1]
    # the s = 512 fold edge term: real[k] += (-1)^k * x[:, :, 512]
    alt = np.where(np.arange(1, S) % 2 == 0, np.float32(1.0), np.float32(-1.0))
    fv[:, :, 2::2] += x[:, :, 512:513] * alt[None, None, :]
    return out
